# revision 8
# baseline (speedup 1.0000x reference)
"""Trainium2 Bass kernel for nn_ConvShare: multi-width causal conv + shared projection.

Reference computation (per batch element b):
    xpad = pad(x[b], L -> L+W-1)                       # [L+11, D]
    taps[k]  = xpad[k:k+L, :] @ conv_w[:, :, k].T      # [L, D], k = 0..W-1
    spans[k] = cumsum_k taps                           # [L, D]
    h[k]     = relu(spans[k])
    out[:, k, :] = h[k] @ proj_w.T + proj_b            # [L, W, D]

Sharding: data-parallel over batch B=8 across the 8 NeuronCores (no
communication; conv_w/proj_w replicated per core).

The kernel is PE-bound: 2 * W * L * D * D = 7.25 GMAC/core = 864 N=512
fp16 matmuls ~= 186.5 us at the warm 2.4 GHz rate. Design:

  - WARMUP dependency-free N=128 matmuls open the HAM clock gate
    (1.2 -> 2.4 GHz after ~3.4us of continuous PE busy) while startup
    DMAs land; startup load choreography unchanged from the tuned
    baseline (split first cw chunk + xt0 on the Scalar queue, per-chunk
    Sync loads in deadline order, pw/pb on Scalar).
  - LAGGED PROJ pipeline: proj of span k-1 is emitted between conv tap
    k and conv tap k+1, so every proj's relu (and fp8 cast) inputs are
    produced a full conv-tap (~6us) earlier - no relu/cast latency can
    ever bubble the PE stream.
  - PSUM cumsum: 6 persistent PSUM banks accumulate the conv across
    taps (start at tap 0, stop at tap 11); relus alternate Scalar/DVE.
  - fp8 (e4m3) DoubleRow matmuls (~1.44x at N=512) for a planned subset
    of conv (tap, chunk-pair) and proj (span, chunk-pair) items, chosen
    offline with an exact numerics model (errmodel/search2.py) against
    the 2e-2 max-rel-err gate. Late conv taps are cheap (error touches
    few spans); early proj spans are cheap (|h| grows ~sqrt(k)).
  - Scale-split quantization: conv fp8 operands are x*2^-3 and cw*2^3
    (product unscaled -> PSUM-compatible with fp16 taps; lifts the
    mostly-subnormal cw values into e4m3's normal range). proj weights
    are pw*2^4 in BOTH fp16 and fp8 copies; every span's bias-add is a
    DVE tensor_scalar (psum * 2^-4 + pb), exact for fp16 and fixing
    e4m3 subnormal waste for fp8.
  - h8 casts: relus always write fp16 h tiles (fast path); the fp8
    copies for fp8-proj spans are produced by GpSimd (otherwise idle)
    as tensor_scalar_sub(h - mu) into interleaved [P, 2, L] pair tiles.
    mu = ALPHA * sigma_span (exact per-d span sigma from conv_w) centers
    relu's one-sided distribution to cut e4m3 cast noise; the exact
    correction mu @ pw8 is folded into the per-span bias host-side.
  - Output is written feature-major ([W, D, L] in DRAM, host transposes
    to [L, W, D] - free for HW time), keeping every matmul at N=512.
"""

import os
import sys

import numpy as np

if True:  # make concourse importable regardless of harness cwd
    for _p in ("/opt/trn_rl_repo", "/opt/pypackages"):
        if _p not in sys.path and os.path.isdir(_p):
            sys.path.append(_p)

from contextlib import ExitStack  # noqa: E402

import ml_dtypes  # noqa: E402

import concourse.bacc as bacc  # noqa: E402
import concourse.bass as bass  # noqa: E402
import concourse.mybir as mybir  # noqa: E402
import concourse.tile as tile  # noqa: E402
from concourse import bass_utils  # noqa: E402

B, L, D, W = 8, 512, 768, 12
P = 128          # SBUF partitions
C = D // P       # 6 contraction chunks of 128
NP = C // 2      # 3 canonical chunk-pairs
LP = L + W - 1   # 523: right-padded sequence length

F32 = mybir.dt.float32
F16 = mybir.dt.float16
F8 = mybir.dt.float8e4
DR = mybir.MatmulPerfMode.DoubleRow
RELU = mybir.ActivationFunctionType.Relu

# ---- fp8 plan (from search2.py exact-model search) -------------------------
# tap/span -> tuple of canonical pair indices (pair g = chunks 2g, 2g+1)
CONV_FP8 = {9: (0, 1, 2), 10: (0, 1), 11: (0, 1, 2)}
PROJ_FP8 = {0: (0, 1, 2), 1: (0, 1, 2), 2: (0, 1, 2), 3: (0, 1), 4: (1,),
            5: (2,), 6: (1,), 7: (0,), 8: (1,)}
CONV_SCALE = 3     # xT8 = f8(x * 2^-3), cw8 = f8(cw * 2^3)
PROJ_SCALE = 4     # pw16/pw8 = pw * 2^4; bias op descales psum by 2^-4
ALPHA = 0.35       # h-centering: h8 = f8(h - ALPHA * sigma_span)
WARMUP = 42
SPLIT_FIRST = True
FP8_LOAD_AT = 5    # iteration that issues the xT8/cw8 bulk loads

# Knobs the test harness may flip before calling kernel():
TRACE = False
LAST_RESULTS = None


def _plan_key():
    return (tuple(sorted((k, tuple(v)) for k, v in CONV_FP8.items())),
            tuple(sorted((k, tuple(v)) for k, v in PROJ_FP8.items())),
            CONV_SCALE, PROJ_SCALE, ALPHA, WARMUP, SPLIT_FIRST, FP8_LOAD_AT)


def _conv8_items():
    return [(k, g) for k in sorted(CONV_FP8) for g in sorted(CONV_FP8[k])]


def _build_program() -> bass.Bass:
    conv8 = _conv8_items()
    cw8_idx = {kg: i for i, kg in enumerate(conv8)}
    any_proj8 = bool(PROJ_FP8)

    nc = bacc.Bacc(
        "TRN2",
        target_bir_lowering=False,
        debug=False,
        num_devices=B,
    )

    # DRAM I/O. Matmul inputs are pre-chunked host-side to [C, P, n] so each
    # chunk DMA is a clean 2D copy and compute can start on chunk 0 early.
    # All bulk operands are partition-major in DRAM so every load is one
    # long contiguous descriptor per partition (~9KB) - small gather
    # descriptors halve effective DMA bandwidth in the startup crunch.
    xT = nc.dram_tensor("xT", [P, C, LP], F16, kind="ExternalInput").ap()
    cw = nc.dram_tensor("cw", [W, P, C, D], F16, kind="ExternalInput").ap()
    pw = nc.dram_tensor("pw", [P, C, D], F16, kind="ExternalInput").ap()
    pb = nc.dram_tensor("pb", [P, W * C], F32, kind="ExternalInput").ap()
    if conv8:
        xT8 = nc.dram_tensor("xT8", [P, NP, 2, LP], F8, kind="ExternalInput").ap()
        cw8 = nc.dram_tensor("cw8", [P, len(conv8), 2, D], F8, kind="ExternalInput").ap()
    if any_proj8:
        pw8 = nc.dram_tensor("pw8", [P, NP, 2, D], F8, kind="ExternalInput").ap()
        mu = nc.dram_tensor("mu", [P, W * C], F32, kind="ExternalInput").ap()
    out = nc.dram_tensor("out", [W, D, L], F32, kind="ExternalOutput").ap()

    with tile.TileContext(nc) as tc, ExitStack() as ctx:
        const_pool = ctx.enter_context(tc.tile_pool(name="const", bufs=1))
        cw_pool = ctx.enter_context(tc.tile_pool(name="cw", bufs=2))
        h_pool = ctx.enter_context(tc.tile_pool(name="h", bufs=2))
        h8_pool = ctx.enter_context(tc.tile_pool(name="h8", bufs=2))
        out_pool = ctx.enter_context(tc.tile_pool(name="out", bufs=4))
        psc_pool = ctx.enter_context(tc.tile_pool(name="psc", bufs=1, space="PSUM"))
        psp_pool = ctx.enter_context(tc.tile_pool(name="psp", bufs=2, space="PSUM"))

        def dma_in(dst_ap, src_ap):
            nc.sync.dma_start(dst_ap, src_ap)

        if WARMUP:
            # Dependency-free matmuls keep the PE continuously busy from the
            # earliest possible moment so the HAM clock gate opens before the
            # real stream takes over.
            wa = const_pool.tile([P, P], F16, name="warm_a")
            nc.vector.memset(wa[:], 0.0)
            wp = psp_pool.tile([P, 512], F32, tag="psp", name="warm_ps")
            for wi in range(WARMUP):
                nc.tensor.matmul(
                    wp[:, 0:P], lhsT=wa[:], rhs=wa[:], start=True, stop=True
                )

        # --- startup loads, critical-path first -------------------------
        # The three tiny loads the first conv matmuls need go on the (empty)
        # Scalar queue; remaining startup loads are per-chunk DMAs on Sync in
        # compute-deadline order. pw/pb/pw8/mu ride the otherwise-idle Scalar
        # queue (first deadline ~21us: the span-0 proj).
        if SPLIT_FIRST:
            cw00a = cw_pool.tile([P, P], F16, tag="cw00a", name="cw00a")
            nc.scalar.dma_start(cw00a[:], cw[0, :, 0, 0:P])
            xt0 = const_pool.tile([P, LP], F16, tag="xt0", name="xt0")
            nc.scalar.dma_start(xt0[:], xT[:, 0, :])
            cw00b = cw_pool.tile([P, D - P], F16, tag="cw00b", name="cw00b")
            nc.scalar.dma_start(cw00b[:], cw[0, :, 0, P:D])
            first = 1
        else:
            first = 0
        cw0_c = [None] * C
        xt_c = [None] * C
        for c in range(first, C):
            t = cw_pool.tile([P, D], F16, tag=f"cw0_{c}", name=f"cw0_{c}")
            dma_in(t[:], cw[0, :, c, :])
            cw0_c[c] = t
            xt = const_pool.tile([P, LP], F16, tag=f"xt{c}", name=f"xt{c}")
            dma_in(xt[:], xT[:, c, :])
            xt_c[c] = xt

        def xt_ap(c):
            if SPLIT_FIRST and c == 0:
                return xt0[:]
            return xt_c[c][:]

        pw_all = const_pool.tile([P, C, D], F16, tag="pw", name="pw")
        nc.scalar.dma_start(pw_all[:], pw[:, :, :])
        pw_t = [pw_all[:, c, :] for c in range(C)]
        pb_t = const_pool.tile([P, W * C], F32, name="pb")
        nc.scalar.dma_start(pb_t[:], pb[:])
        if any_proj8:
            pw8_t = const_pool.tile([P, NP, 2, D], F8, name="pw8")
            nc.scalar.dma_start(pw8_t[:], pw8[:, :, :, :])
            mu_t = const_pool.tile([P, W * C], F32, name="mu")
            nc.scalar.dma_start(mu_t[:], mu[:])
        if conv8:
            xT8_t = const_pool.tile([P, NP, 2, LP], F8, name="xT8")
            cw8_t = const_pool.tile([P, len(conv8), 2, D], F8, name="cw8")

            def load_fp8():
                dma_in(xT8_t[:], xT8[:, :, :, :])
                dma_in(cw8_t[:], cw8[:, :, :, :])

        cw_tiles = {}

        def tap_f16_chunks(k):
            cov = {c for g in CONV_FP8.get(k, ()) for c in (2 * g, 2 * g + 1)}
            return [c for c in range(C) if c not in cov]

        def load_cw(k):
            # Per-chunk DMAs into separate tiles: each rides its own DMA
            # channel (a single consolidated load is one channel and takes
            # ~8us - it stalled conv tap 1 by ~7us). Only the chunks the
            # fp16 matmuls actually need are loaded.
            tiles = {}
            for c in tap_f16_chunks(k):
                t = cw_pool.tile([P, D], F16, tag=f"cwt{c}", name=f"cw_{k}_{c}")
                dma_in(t[:], cw[k, :, c, :])
                tiles[c] = t
            return tiles

        def cw_slice(k, c, ob):
            """lhsT [P, 128] for conv tap k, contraction chunk c, out block ob."""
            if k == 0:
                if SPLIT_FIRST and c == 0:
                    if ob == 0:
                        return cw00a[:]
                    return cw00b[:, (ob - 1) * P: ob * P]
                return cw0_c[c][:, ob * P: (ob + 1) * P]
            return cw_tiles[k][c][:, ob * P: (ob + 1) * P]

        # 6 persistent PSUM banks accumulate the conv cumsum across taps.
        sp_acc = [
            psc_pool.tile([P, L], F32, tag=f"sp{ob}", name=f"sp{ob}")
            for ob in range(C)
        ]

        def fp8_cov(k):
            return {c: g for g in PROJ_FP8.get(k, ()) for c in (2 * g, 2 * g + 1)}

        def emit_post_chunk(k, ob, h_cur, h8_cur):
            cov = fp8_cov(k)
            if ob in cov:
                # Fused relu + centering + fp8 cast, straight from PSUM, on
                # the DVE: h8 = relu(psum) - mu. The fp16 copy of this chunk
                # is unused by this span's proj, so no separate relu needed.
                g = cov[ob]
                nc.vector.tensor_scalar(
                    out=h8_cur[g][:, (ob - 2 * g), :], in0=sp_acc[ob][:],
                    scalar1=0.0,
                    scalar2=mu_t[:, k * C + ob: k * C + ob + 1],
                    op0=mybir.AluOpType.max, op1=mybir.AluOpType.subtract,
                )
            elif len(cov) >= 2 or ob % 2 == 0:
                nc.scalar.activation(h_cur[ob][:], sp_acc[ob][:], RELU)
            else:
                nc.vector.tensor_scalar_max(h_cur[ob][:], sp_acc[ob][:], 0.0)

        def new_h_tiles(k):
            cov = fp8_cov(k)
            h_cur = [None if c in cov else
                     h_pool.tile([P, L], F16, tag=f"h{c}", name=f"h{c}_{k}")
                     for c in range(C)]
            h8_cur = {g: h8_pool.tile([P, 2, L], F8, tag=f"h8_{g}", name=f"h8_{g}_{k}")
                      for g in PROJ_FP8.get(k, ())}
            return h_cur, h8_cur

        def emit_conv_tap(k, h_cur, h8_cur):
            pairs8 = sorted(CONV_FP8.get(k, ()))
            f16c = tap_f16_chunks(k)
            for ob in range(C):
                ops = [("8", g) for g in pairs8] + [("f", c) for c in f16c]
                for idx, (t, v) in enumerate(ops):
                    last = (k == W - 1) and idx == len(ops) - 1
                    if t == "8":
                        nc.tensor.matmul(
                            sp_acc[ob][:],
                            lhsT=cw8_t[:, cw8_idx[(k, v)], :, ob * P:(ob + 1) * P],
                            rhs=xT8_t[:, v, :, k: k + L],
                            start=False, stop=last, perf_mode=DR,
                            skip_group_check=True,
                        )
                    else:
                        nc.tensor.matmul(
                            sp_acc[ob][:],
                            lhsT=cw_slice(k, v, ob),
                            rhs=xt_ap(v)[:, k: k + L],
                            start=False, stop=last,
                            skip_group_check=True,
                        )
                emit_post_chunk(k, ob, h_cur, h8_cur)

        def emit_proj(s, h_prev, h8_prev):
            pairs8 = sorted(PROJ_FP8.get(s, ()))
            cov = {c for g in pairs8 for c in (2 * g, 2 * g + 1)}
            f16c = [c for c in range(C) if c not in cov]
            for o2b in range(C):
                pp = psp_pool.tile([P, 512], F32, tag="psp", name=f"pp_{s}_{o2b}")
                ops = [("8", g) for g in pairs8] + [("f", c) for c in f16c]
                for idx, (t, v) in enumerate(ops):
                    if t == "8":
                        nc.tensor.matmul(
                            pp[:],
                            lhsT=pw8_t[:, v, :, o2b * P:(o2b + 1) * P],
                            rhs=h8_prev[v][:],
                            start=(idx == 0), stop=(idx == len(ops) - 1),
                            perf_mode=DR,
                        )
                    else:
                        nc.tensor.matmul(
                            pp[:],
                            lhsT=pw_t[v][:, o2b * P:(o2b + 1) * P],
                            rhs=h_prev[v][:],
                            start=(idx == 0), stop=(idx == len(ops) - 1),
                        )
                o_t = out_pool.tile([P, L], F32, tag="out", name=f"o_{s}_{o2b}")
                # Bias + 2^-PROJ_SCALE descale; split DVE/ACT to balance load
                # (Identity is in every ACT table set - no table reload).
                if o2b % 2 == 0:
                    nc.vector.tensor_scalar(
                        out=o_t[:], in0=pp[:],
                        scalar1=2.0 ** -PROJ_SCALE,
                        scalar2=pb_t[:, s * C + o2b: s * C + o2b + 1],
                        op0=mybir.AluOpType.mult, op1=mybir.AluOpType.add,
                    )
                else:
                    nc.scalar.activation(
                        o_t[:], pp[:], mybir.ActivationFunctionType.Identity,
                        bias=pb_t[:, s * C + o2b: s * C + o2b + 1],
                        scale=2.0 ** -PROJ_SCALE,
                    )
                nc.sync.dma_start(out[s, o2b * P:(o2b + 1) * P, :], o_t[:])

        # ---- tap 0: c-outer so contraction chunk c is needed only at
        # conv_start + c*1.28us, matching HBM arrival. -------------------
        h_cur, h8_cur = new_h_tiles(0)
        for c in range(C):
            for ob in range(C):
                nc.tensor.matmul(
                    sp_acc[ob][:],
                    lhsT=cw_slice(0, c, ob),
                    rhs=xt_ap(c)[:, 0:L],
                    start=(c == 0), stop=False,
                    skip_group_check=True,
                )
        for ob in range(C):
            emit_post_chunk(0, ob, h_cur, h8_cur)

        # ---- main lagged loop ------------------------------------------
        if 1 < W and tap_f16_chunks(1):
            cw_tiles[1] = load_cw(1)
        h_prev, h8_prev = h_cur, h8_cur
        for k in range(1, W):
            if k + 1 < W and tap_f16_chunks(k + 1):
                cw_tiles[k + 1] = load_cw(k + 1)
            if k == FP8_LOAD_AT and conv8:
                load_fp8()
            h_cur, h8_cur = new_h_tiles(k)
            emit_conv_tap(k, h_cur, h8_cur)
            emit_proj(k - 1, h_prev, h8_prev)
            h_prev, h8_prev = h_cur, h8_cur
        emit_proj(W - 1, h_prev, h8_prev)

    nc.compile()
    return nc


_program_cache: dict = {}


def _get_program() -> bass.Bass:
    key = _plan_key()
    if key not in _program_cache:
        _program_cache[key] = _build_program()
    return _program_cache[key]


def _prep_inputs(x, conv_w, proj_w, proj_b):
    x = np.asarray(x, dtype=np.float32)
    conv_w = np.asarray(conv_w, dtype=np.float32)
    proj_w = np.asarray(proj_w, dtype=np.float32)
    proj_b = np.asarray(proj_b, dtype=np.float32)
    f8 = ml_dtypes.float8_e4m3

    xT_f32 = np.zeros((B, D, LP), dtype=np.float32)              # [B, D, L+W-1]
    xT_f32[:, :, :L] = x.transpose(0, 2, 1)
    xT_f32 = xT_f32.reshape(B, C, P, LP)
    xT16 = np.ascontiguousarray(xT_f32.transpose(0, 2, 1, 3).astype(np.float16))
    cwT_f32 = conv_w.transpose(2, 1, 0).reshape(W, C, P, D)      # [W, C, P, o]
    cw16 = np.ascontiguousarray(cwT_f32.transpose(0, 2, 1, 3).astype(np.float16))
    pw_f32 = proj_w.T.reshape(C, P, D) * 2.0 ** PROJ_SCALE
    pw16 = np.ascontiguousarray(pw_f32.transpose(1, 0, 2).astype(np.float16))

    maps = {"xT": xT16, "cw": cw16, "pw": pw16}
    per_b = {"xT"}

    conv8 = _conv8_items()
    if conv8:
        # Pair layout [g, P, 2, n] feeds DoubleRow matmuls (contract 2
        # k-chunks per instruction). Scale-split: product is unscaled.
        maps["xT8"] = np.ascontiguousarray(
            (xT_f32 * 2.0 ** -CONV_SCALE)
            .reshape(B, NP, 2, P, LP).transpose(0, 3, 1, 2, 4).astype(f8))
        per_b.add("xT8")
        cw8 = np.stack([
            (cwT_f32[k, 2 * g: 2 * g + 2] * 2.0 ** CONV_SCALE).transpose(1, 0, 2)
            for (k, g) in conv8
        ])  # [n_items, P, 2, D]
        maps["cw8"] = np.ascontiguousarray(cw8.transpose(1, 0, 2, 3).astype(f8))

    # per-span bias, with the h-centering correction folded in
    pb_full = np.broadcast_to(proj_b, (W, D)).copy()             # [W, D(o)]
    if PROJ_FP8:
        pw8_pairs = pw_f32.reshape(NP, 2, P, D).transpose(0, 2, 1, 3)
        pw8_q = pw8_pairs.astype(f8)
        maps["pw8"] = np.ascontiguousarray(pw8_q.transpose(1, 0, 2, 3))
        # sigma of span s per input-dim d (x ~ iid N(0,1) exactly known)
        sig = np.sqrt(np.cumsum((conv_w ** 2).sum(axis=1), axis=-1)).T  # [W, D]
        mu_f = ALPHA * sig                                        # [W, D_in]
        mu_t = np.zeros((P, W * C), np.float32)
        pw8_f32 = (pw8_q.astype(np.float32)
                   .transpose(0, 2, 1, 3).reshape(D, D)) * 2.0 ** -PROJ_SCALE
        for s, pairs in PROJ_FP8.items():
            chunks = [c for g in pairs for c in (2 * g, 2 * g + 1)]
            for c in chunks:
                mu_t[:, s * C + c] = mu_f[s, c * P:(c + 1) * P]
            dmask = np.zeros(D, np.float32)
            for c in chunks:
                dmask[c * P:(c + 1) * P] = 1.0
            pb_full[s] += (mu_f[s] * dmask) @ pw8_f32
        maps["mu"] = np.ascontiguousarray(mu_t)
    pb_t = np.zeros((P, W * C), np.float32)
    for s in range(W):
        pb_t[:, s * C:(s + 1) * C] = pb_full[s].reshape(C, P).T
    maps["pb"] = np.ascontiguousarray(pb_t)
    return maps, per_b


def kernel(x, conv_w, proj_w, proj_b):
    global LAST_RESULTS
    nc = _get_program()
    maps, per_b = _prep_inputs(x, conv_w, proj_w, proj_b)
    in_maps = []
    for b in range(B):
        in_maps.append({k: (v[b] if k in per_b else v) for k, v in maps.items()})
    res = bass_utils.run_bass_kernel_spmd(
        nc, in_maps, core_ids=list(range(B)), trace=TRACE
    )
    LAST_RESULTS = res
    # per-core out is [W, D, L]; final layout is [L, W, D]
    return np.stack(
        [np.ascontiguousarray(r["out"].transpose(2, 0, 1)) for r in res.results],
        axis=0,
    )


# revision 10
# speedup vs baseline: 1.1188x; 1.1188x over previous
"""Trainium2 Bass kernel for nn_ConvShare: multi-width causal conv + shared projection.

Reference computation (per batch element b):
    xpad = pad(x[b], L -> L+W-1)                       # [L+11, D]
    taps[k]  = xpad[k:k+L, :] @ conv_w[:, :, k].T      # [L, D], k = 0..W-1
    spans[k] = cumsum_k taps                           # [L, D]
    h[k]     = relu(spans[k])
    out[:, k, :] = h[k] @ proj_w.T + proj_b            # [L, W, D]

Sharding: data-parallel over batch B=8 across the 8 NeuronCores (no
communication; conv_w/proj_w replicated per core).

The kernel is PE-bound: 2 * W * L * D * D = 7.25 GMAC/core = 864 N=512
fp16 matmuls ~= 186.5 us at the warm 2.4 GHz rate. Design:

  - WARMUP dependency-free N=128 matmuls open the HAM clock gate
    (1.2 -> 2.4 GHz after ~3.4us of continuous PE busy) while startup
    DMAs land; startup load choreography unchanged from the tuned
    baseline (split first cw chunk + xt0 on the Scalar queue, per-chunk
    Sync loads in deadline order, pw/pb on Scalar).
  - LAGGED PROJ pipeline: proj of span k-1 is emitted between conv tap
    k and conv tap k+1, so every proj's relu (and fp8 cast) inputs are
    produced a full conv-tap (~6us) earlier - no relu/cast latency can
    ever bubble the PE stream.
  - PSUM cumsum: 6 persistent PSUM banks accumulate the conv across
    taps (start at tap 0, stop at tap 11); relus alternate Scalar/DVE.
  - fp8 (e4m3) DoubleRow matmuls (~1.44x at N=512) for a planned subset
    of conv (tap, chunk-pair) and proj (span, chunk-pair) items, chosen
    offline with an exact numerics model (errmodel/search2.py) against
    the 2e-2 max-rel-err gate. Late conv taps are cheap (error touches
    few spans); early proj spans are cheap (|h| grows ~sqrt(k)).
  - Scale-split quantization: conv fp8 operands are x*2^-3 and cw*2^3
    (product unscaled -> PSUM-compatible with fp16 taps; lifts the
    mostly-subnormal cw values into e4m3's normal range). proj weights
    are pw*2^4 in BOTH fp16 and fp8 copies; every span's bias-add is a
    DVE tensor_scalar (psum * 2^-4 + pb), exact for fp16 and fixing
    e4m3 subnormal waste for fp8.
  - h8 casts: relus always write fp16 h tiles (fast path); the fp8
    copies for fp8-proj spans are produced by GpSimd (otherwise idle)
    as tensor_scalar_sub(h - mu) into interleaved [P, 2, L] pair tiles.
    mu = ALPHA * sigma_span (exact per-d span sigma from conv_w) centers
    relu's one-sided distribution to cut e4m3 cast noise; the exact
    correction mu @ pw8 is folded into the per-span bias host-side.
  - Output is written feature-major ([W, D, L] in DRAM, host transposes
    to [L, W, D] - free for HW time), keeping every matmul at N=512.
"""

import os
import sys

import numpy as np

if True:  # make concourse importable regardless of harness cwd
    for _p in ("/opt/trn_rl_repo", "/opt/pypackages"):
        if _p not in sys.path and os.path.isdir(_p):
            sys.path.append(_p)

from contextlib import ExitStack  # noqa: E402

import ml_dtypes  # noqa: E402

import concourse.bacc as bacc  # noqa: E402
import concourse.bass as bass  # noqa: E402
import concourse.mybir as mybir  # noqa: E402
import concourse.tile as tile  # noqa: E402
from concourse import bass_utils  # noqa: E402

B, L, D, W = 8, 512, 768, 12
P = 128          # SBUF partitions
C = D // P       # 6 contraction chunks of 128
NP = C // 2      # 3 canonical chunk-pairs
LP = L + W - 1   # 523: right-padded sequence length

F32 = mybir.dt.float32
F16 = mybir.dt.float16
F8 = mybir.dt.float8e4
DR = mybir.MatmulPerfMode.DoubleRow
RELU = mybir.ActivationFunctionType.Relu

# ---- fp8 plan (from search2.py exact-model search) -------------------------
# tap/span -> tuple of canonical pair indices (pair g = chunks 2g, 2g+1)
CONV_FP8 = {9: (0, 1, 2), 10: (0, 1), 11: (0, 1, 2)}
PROJ_FP8 = {0: (0, 1, 2), 1: (0, 1, 2), 2: (0, 1, 2), 3: (0, 1), 4: (1,),
            5: (2,), 6: (1,), 7: (0,), 8: (1,)}
CONV_SCALE = 3     # xT8 = f8(x * 2^-3), cw8 = f8(cw * 2^3)
PROJ_SCALE = 4     # pw16/pw8 = pw * 2^4; bias op descales psum by 2^-4
ALPHA = 0.35       # h-centering: h8 = f8(h - ALPHA * sigma_span)
WARMUP = 42
SPLIT_FIRST = True
FP8_LOAD_AT = 5    # iteration that issues the xT8/cw8 bulk loads

# Knobs the test harness may flip before calling kernel():
TRACE = False
LAST_RESULTS = None


def _plan_key():
    return (tuple(sorted((k, tuple(v)) for k, v in CONV_FP8.items())),
            tuple(sorted((k, tuple(v)) for k, v in PROJ_FP8.items())),
            CONV_SCALE, PROJ_SCALE, ALPHA, WARMUP, SPLIT_FIRST, FP8_LOAD_AT)


def _conv8_items():
    return [(k, g) for k in sorted(CONV_FP8) for g in sorted(CONV_FP8[k])]


def _build_program() -> bass.Bass:
    conv8 = _conv8_items()
    cw8_idx = {kg: i for i, kg in enumerate(conv8)}
    any_proj8 = bool(PROJ_FP8)

    nc = bacc.Bacc(
        "TRN2",
        target_bir_lowering=False,
        debug=False,
        num_devices=B,
    )

    # DRAM I/O. Matmul inputs are pre-chunked host-side to [C, P, n] so each
    # chunk DMA is a clean 2D copy and compute can start on chunk 0 early.
    # All bulk operands are partition-major in DRAM so every load is one
    # long contiguous descriptor per partition (~9KB) - small gather
    # descriptors halve effective DMA bandwidth in the startup crunch.
    xT = nc.dram_tensor("xT", [P, C, LP], F16, kind="ExternalInput").ap()
    cw = nc.dram_tensor("cw", [W, P, C, D], F16, kind="ExternalInput").ap()
    pw = nc.dram_tensor("pw", [P, C, D], F16, kind="ExternalInput").ap()
    pb = nc.dram_tensor("pb", [P, W * C], F32, kind="ExternalInput").ap()
    if conv8:
        xT8 = nc.dram_tensor("xT8", [P, NP, 2, LP], F8, kind="ExternalInput").ap()
        cw8 = nc.dram_tensor("cw8", [P, len(conv8), 2, D], F8, kind="ExternalInput").ap()
    if any_proj8:
        pw8 = nc.dram_tensor("pw8", [P, NP, 2, D], F8, kind="ExternalInput").ap()
        mu = nc.dram_tensor("mu", [P, W * C], F32, kind="ExternalInput").ap()
    out = nc.dram_tensor("out", [W, D, L], F32, kind="ExternalOutput").ap()

    with tile.TileContext(nc) as tc, ExitStack() as ctx:
        const_pool = ctx.enter_context(tc.tile_pool(name="const", bufs=1))
        cw_pool = ctx.enter_context(tc.tile_pool(name="cw", bufs=2))
        h_pool = ctx.enter_context(tc.tile_pool(name="h", bufs=2))
        h8_pool = ctx.enter_context(tc.tile_pool(name="h8", bufs=2))
        out_pool = ctx.enter_context(tc.tile_pool(name="out", bufs=4))
        psc_pool = ctx.enter_context(tc.tile_pool(name="psc", bufs=1, space="PSUM"))
        psp_pool = ctx.enter_context(tc.tile_pool(name="psp", bufs=2, space="PSUM"))

        def dma_in(dst_ap, src_ap):
            nc.sync.dma_start(dst_ap, src_ap)

        if WARMUP:
            # Dependency-free matmuls keep the PE continuously busy from the
            # earliest possible moment so the HAM clock gate opens before the
            # real stream takes over.
            wa = const_pool.tile([P, P], F16, name="warm_a")
            nc.vector.memset(wa[:], 0.0)
            wp = psp_pool.tile([P, 512], F32, tag="psp", name="warm_ps")
            for wi in range(WARMUP):
                nc.tensor.matmul(
                    wp[:, 0:P], lhsT=wa[:], rhs=wa[:], start=True, stop=True
                )

        # --- startup loads, critical-path first -------------------------
        # The three tiny loads the first conv matmuls need go on the (empty)
        # Scalar queue; remaining startup loads are per-chunk DMAs on Sync in
        # compute-deadline order. pw/pb/pw8/mu ride the otherwise-idle Scalar
        # queue (first deadline ~21us: the span-0 proj).
        if SPLIT_FIRST:
            cw00a = cw_pool.tile([P, P], F16, tag="cw00a", name="cw00a")
            nc.scalar.dma_start(cw00a[:], cw[0, :, 0, 0:P])
            xt0 = const_pool.tile([P, LP], F16, tag="xt0", name="xt0")
            nc.scalar.dma_start(xt0[:], xT[:, 0, :])
            cw00b = cw_pool.tile([P, D - P], F16, tag="cw00b", name="cw00b")
            nc.scalar.dma_start(cw00b[:], cw[0, :, 0, P:D])
            first = 1
        else:
            first = 0
        cw0_c = [None] * C
        xt_c = [None] * C
        for c in range(first, C):
            t = cw_pool.tile([P, D], F16, tag=f"cw0_{c}", name=f"cw0_{c}")
            dma_in(t[:], cw[0, :, c, :])
            cw0_c[c] = t
            xt = const_pool.tile([P, LP], F16, tag=f"xt{c}", name=f"xt{c}")
            dma_in(xt[:], xT[:, c, :])
            xt_c[c] = xt

        def xt_ap(c):
            if SPLIT_FIRST and c == 0:
                return xt0[:]
            return xt_c[c][:]

        def tap_f16_chunks(k):
            cov = {c for g in CONV_FP8.get(k, ()) for c in (2 * g, 2 * g + 1)}
            return [c for c in range(C) if c not in cov]

        def load_cw(k, scalar_q=False):
            # Per-chunk DMAs into separate tiles: each rides its own DMA
            # channel (a single consolidated load is one channel and takes
            # ~8us - it stalled conv tap 1 by ~7us). Only the chunks the
            # fp16 matmuls actually need are loaded.
            tiles = {}
            for c in tap_f16_chunks(k):
                t = cw_pool.tile([P, D], F16, tag=f"cwt{c}", name=f"cw_{k}_{c}")
                if scalar_q:
                    nc.scalar.dma_start(t[:], cw[k, :, c, :])
                else:
                    dma_in(t[:], cw[k, :, c, :])
                tiles[c] = t
            return tiles

        cw_tiles = {}

        # cw tap 1 rides the Scalar queue AHEAD of pw/pb (those have ~30us
        # of slack; conv tap 1 needs cw1 by ~13us and the Sync queue is
        # busy with the conv-0 chunk loads).
        if tap_f16_chunks(1):
            cw_tiles[1] = load_cw(1, scalar_q=True)

        pw_all = const_pool.tile([P, C, D], F16, tag="pw", name="pw")
        nc.scalar.dma_start(pw_all[:], pw[:, :, :])
        pw_t = [pw_all[:, c, :] for c in range(C)]
        pb_t = const_pool.tile([P, W * C], F32, name="pb")
        nc.scalar.dma_start(pb_t[:], pb[:])
        if any_proj8:
            pw8_t = const_pool.tile([P, NP, 2, D], F8, name="pw8")
            nc.scalar.dma_start(pw8_t[:], pw8[:, :, :, :])
            mu_t = const_pool.tile([P, W * C], F32, name="mu")
            nc.scalar.dma_start(mu_t[:], mu[:])
        if conv8:
            xT8_t = const_pool.tile([P, NP, 2, LP], F8, name="xT8")
            cw8_t = const_pool.tile([P, len(conv8), 2, D], F8, name="cw8")

            def load_fp8():
                dma_in(xT8_t[:], xT8[:, :, :, :])
                dma_in(cw8_t[:], cw8[:, :, :, :])

        def cw_slice(k, c, ob):
            """lhsT [P, 128] for conv tap k, contraction chunk c, out block ob."""
            if k == 0:
                if SPLIT_FIRST and c == 0:
                    if ob == 0:
                        return cw00a[:]
                    return cw00b[:, (ob - 1) * P: ob * P]
                return cw0_c[c][:, ob * P: (ob + 1) * P]
            return cw_tiles[k][c][:, ob * P: (ob + 1) * P]

        # 6 persistent PSUM banks accumulate the conv cumsum across taps.
        sp_acc = [
            psc_pool.tile([P, L], F32, tag=f"sp{ob}", name=f"sp{ob}")
            for ob in range(C)
        ]

        def fp8_cov(k):
            return {c: g for g in PROJ_FP8.get(k, ()) for c in (2 * g, 2 * g + 1)}

        def emit_post_chunk(k, ob, h_cur, h8_cur):
            cov = fp8_cov(k)
            if ob in cov:
                # Fused relu + centering + fp8 cast, straight from PSUM, on
                # the DVE: h8 = relu(psum) - mu. The fp16 copy of this chunk
                # is unused by this span's proj, so no separate relu needed.
                g = cov[ob]
                nc.vector.tensor_scalar(
                    out=h8_cur[g][:, (ob - 2 * g), :], in0=sp_acc[ob][:],
                    scalar1=0.0,
                    scalar2=mu_t[:, k * C + ob: k * C + ob + 1],
                    op0=mybir.AluOpType.max, op1=mybir.AluOpType.subtract,
                )
            elif len(cov) >= 2 or ob % 2 == 0:
                nc.scalar.activation(h_cur[ob][:], sp_acc[ob][:], RELU)
            else:
                nc.vector.tensor_scalar_max(h_cur[ob][:], sp_acc[ob][:], 0.0)

        def new_h_tiles(k):
            cov = fp8_cov(k)
            h_cur = [None if c in cov else
                     h_pool.tile([P, L], F16, tag=f"h{c}", name=f"h{c}_{k}")
                     for c in range(C)]
            h8_cur = {g: h8_pool.tile([P, 2, L], F8, tag=f"h8_{g}", name=f"h8_{g}_{k}")
                      for g in PROJ_FP8.get(k, ())}
            return h_cur, h8_cur

        def emit_conv_tap(k, h_cur, h8_cur):
            pairs8 = sorted(CONV_FP8.get(k, ()))
            f16c = tap_f16_chunks(k)
            for ob in range(C):
                ops = [("8", g) for g in pairs8] + [("f", c) for c in f16c]
                for idx, (t, v) in enumerate(ops):
                    last = (k == W - 1) and idx == len(ops) - 1
                    if t == "8":
                        nc.tensor.matmul(
                            sp_acc[ob][:],
                            lhsT=cw8_t[:, cw8_idx[(k, v)], :, ob * P:(ob + 1) * P],
                            rhs=xT8_t[:, v, :, k: k + L],
                            start=False, stop=last, perf_mode=DR,
                            skip_group_check=True,
                        )
                    else:
                        nc.tensor.matmul(
                            sp_acc[ob][:],
                            lhsT=cw_slice(k, v, ob),
                            rhs=xt_ap(v)[:, k: k + L],
                            start=False, stop=last,
                            skip_group_check=True,
                        )
                emit_post_chunk(k, ob, h_cur, h8_cur)

        def emit_proj(s, h_prev, h8_prev):
            pairs8 = sorted(PROJ_FP8.get(s, ()))
            cov = {c for g in pairs8 for c in (2 * g, 2 * g + 1)}
            f16c = [c for c in range(C) if c not in cov]
            for o2b in range(C):
                pp = psp_pool.tile([P, 512], F32, tag="psp", name=f"pp_{s}_{o2b}")
                ops = [("8", g) for g in pairs8] + [("f", c) for c in f16c]
                for idx, (t, v) in enumerate(ops):
                    if t == "8":
                        nc.tensor.matmul(
                            pp[:],
                            lhsT=pw8_t[:, v, :, o2b * P:(o2b + 1) * P],
                            rhs=h8_prev[v][:],
                            start=(idx == 0), stop=(idx == len(ops) - 1),
                            perf_mode=DR,
                        )
                    else:
                        nc.tensor.matmul(
                            pp[:],
                            lhsT=pw_t[v][:, o2b * P:(o2b + 1) * P],
                            rhs=h_prev[v][:],
                            start=(idx == 0), stop=(idx == len(ops) - 1),
                        )
                o_t = out_pool.tile([P, L], F32, tag="out", name=f"o_{s}_{o2b}")
                # Bias + 2^-PROJ_SCALE descale; split DVE/ACT to balance load
                # (Identity is in every ACT table set - no table reload).
                if o2b % 2 == 0:
                    nc.vector.tensor_scalar(
                        out=o_t[:], in0=pp[:],
                        scalar1=2.0 ** -PROJ_SCALE,
                        scalar2=pb_t[:, s * C + o2b: s * C + o2b + 1],
                        op0=mybir.AluOpType.mult, op1=mybir.AluOpType.add,
                    )
                else:
                    nc.scalar.activation(
                        o_t[:], pp[:], mybir.ActivationFunctionType.Identity,
                        bias=pb_t[:, s * C + o2b: s * C + o2b + 1],
                        scale=2.0 ** -PROJ_SCALE,
                    )
                nc.sync.dma_start(out[s, o2b * P:(o2b + 1) * P, :], o_t[:])

        # ---- tap 0: c-outer so contraction chunk c is needed only at
        # conv_start + c*1.28us, matching HBM arrival. -------------------
        h_cur, h8_cur = new_h_tiles(0)
        for c in range(C):
            for ob in range(C):
                nc.tensor.matmul(
                    sp_acc[ob][:],
                    lhsT=cw_slice(0, c, ob),
                    rhs=xt_ap(c)[:, 0:L],
                    start=(c == 0), stop=False,
                    skip_group_check=True,
                )
        for ob in range(C):
            emit_post_chunk(0, ob, h_cur, h8_cur)

        # ---- main lagged loop ------------------------------------------
        h_prev, h8_prev = h_cur, h8_cur
        for k in range(1, W):
            if k + 1 < W and tap_f16_chunks(k + 1):
                cw_tiles[k + 1] = load_cw(k + 1)
            if k == FP8_LOAD_AT and conv8:
                load_fp8()
            h_cur, h8_cur = new_h_tiles(k)
            emit_conv_tap(k, h_cur, h8_cur)
            emit_proj(k - 1, h_prev, h8_prev)
            h_prev, h8_prev = h_cur, h8_cur
        emit_proj(W - 1, h_prev, h8_prev)

    nc.compile()
    return nc


_program_cache: dict = {}


def _get_program() -> bass.Bass:
    key = _plan_key()
    if key not in _program_cache:
        _program_cache[key] = _build_program()
    return _program_cache[key]


def _prep_inputs(x, conv_w, proj_w, proj_b):
    x = np.asarray(x, dtype=np.float32)
    conv_w = np.asarray(conv_w, dtype=np.float32)
    proj_w = np.asarray(proj_w, dtype=np.float32)
    proj_b = np.asarray(proj_b, dtype=np.float32)
    f8 = ml_dtypes.float8_e4m3

    xT_f32 = np.zeros((B, D, LP), dtype=np.float32)              # [B, D, L+W-1]
    xT_f32[:, :, :L] = x.transpose(0, 2, 1)
    xT_f32 = xT_f32.reshape(B, C, P, LP)
    xT16 = np.ascontiguousarray(xT_f32.transpose(0, 2, 1, 3).astype(np.float16))
    cwT_f32 = conv_w.transpose(2, 1, 0).reshape(W, C, P, D)      # [W, C, P, o]
    cw16 = np.ascontiguousarray(cwT_f32.transpose(0, 2, 1, 3).astype(np.float16))
    pw_f32 = proj_w.T.reshape(C, P, D) * 2.0 ** PROJ_SCALE
    pw16 = np.ascontiguousarray(pw_f32.transpose(1, 0, 2).astype(np.float16))

    maps = {"xT": xT16, "cw": cw16, "pw": pw16}
    per_b = {"xT"}

    conv8 = _conv8_items()
    if conv8:
        # Pair layout [g, P, 2, n] feeds DoubleRow matmuls (contract 2
        # k-chunks per instruction). Scale-split: product is unscaled.
        maps["xT8"] = np.ascontiguousarray(
            (xT_f32 * 2.0 ** -CONV_SCALE)
            .reshape(B, NP, 2, P, LP).transpose(0, 3, 1, 2, 4).astype(f8))
        per_b.add("xT8")
        cw8 = np.stack([
            (cwT_f32[k, 2 * g: 2 * g + 2] * 2.0 ** CONV_SCALE).transpose(1, 0, 2)
            for (k, g) in conv8
        ])  # [n_items, P, 2, D]
        maps["cw8"] = np.ascontiguousarray(cw8.transpose(1, 0, 2, 3).astype(f8))

    # per-span bias, with the h-centering correction folded in
    pb_full = np.broadcast_to(proj_b, (W, D)).copy()             # [W, D(o)]
    if PROJ_FP8:
        pw8_pairs = pw_f32.reshape(NP, 2, P, D).transpose(0, 2, 1, 3)
        pw8_q = pw8_pairs.astype(f8)
        maps["pw8"] = np.ascontiguousarray(pw8_q.transpose(1, 0, 2, 3))
        # sigma of span s per input-dim d (x ~ iid N(0,1) exactly known)
        sig = np.sqrt(np.cumsum((conv_w ** 2).sum(axis=1), axis=-1)).T  # [W, D]
        mu_f = ALPHA * sig                                        # [W, D_in]
        mu_t = np.zeros((P, W * C), np.float32)
        pw8_f32 = (pw8_q.astype(np.float32)
                   .transpose(0, 2, 1, 3).reshape(D, D)) * 2.0 ** -PROJ_SCALE
        for s, pairs in PROJ_FP8.items():
            chunks = [c for g in pairs for c in (2 * g, 2 * g + 1)]
            for c in chunks:
                mu_t[:, s * C + c] = mu_f[s, c * P:(c + 1) * P]
            dmask = np.zeros(D, np.float32)
            for c in chunks:
                dmask[c * P:(c + 1) * P] = 1.0
            pb_full[s] += (mu_f[s] * dmask) @ pw8_f32
        maps["mu"] = np.ascontiguousarray(mu_t)
    pb_t = np.zeros((P, W * C), np.float32)
    for s in range(W):
        pb_t[:, s * C:(s + 1) * C] = pb_full[s].reshape(C, P).T
    maps["pb"] = np.ascontiguousarray(pb_t)
    return maps, per_b


def kernel(x, conv_w, proj_w, proj_b):
    global LAST_RESULTS
    nc = _get_program()
    maps, per_b = _prep_inputs(x, conv_w, proj_w, proj_b)
    in_maps = []
    for b in range(B):
        in_maps.append({k: (v[b] if k in per_b else v) for k, v in maps.items()})
    res = bass_utils.run_bass_kernel_spmd(
        nc, in_maps, core_ids=list(range(B)), trace=TRACE
    )
    LAST_RESULTS = res
    # per-core out is [W, D, L]; final layout is [L, W, D]
    return np.stack(
        [np.ascontiguousarray(r["out"].transpose(2, 0, 1)) for r in res.results],
        axis=0,
    )


# revision 11
# speedup vs baseline: 1.1885x; 1.0623x over previous
"""Trainium2 Bass kernel for nn_ConvShare: multi-width causal conv + shared projection.

Reference computation (per batch element b):
    xpad = pad(x[b], L -> L+W-1)                       # [L+11, D]
    taps[k]  = xpad[k:k+L, :] @ conv_w[:, :, k].T      # [L, D], k = 0..W-1
    spans[k] = cumsum_k taps                           # [L, D]
    h[k]     = relu(spans[k])
    out[:, k, :] = h[k] @ proj_w.T + proj_b            # [L, W, D]

Sharding: data-parallel over batch B=8 across the 8 NeuronCores (no
communication; conv_w/proj_w replicated per core).

The kernel is PE-bound: 2 * W * L * D * D = 7.25 GMAC/core = 864 N=512
fp16 matmuls ~= 186.5 us at the warm 2.4 GHz rate. Design:

  - WARMUP dependency-free N=128 matmuls open the HAM clock gate
    (1.2 -> 2.4 GHz after ~3.4us of continuous PE busy) while startup
    DMAs land; startup load choreography unchanged from the tuned
    baseline (split first cw chunk + xt0 on the Scalar queue, per-chunk
    Sync loads in deadline order, pw/pb on Scalar).
  - LAGGED PROJ pipeline: proj of span k-1 is emitted between conv tap
    k and conv tap k+1, so every proj's relu (and fp8 cast) inputs are
    produced a full conv-tap (~6us) earlier - no relu/cast latency can
    ever bubble the PE stream.
  - PSUM cumsum: 6 persistent PSUM banks accumulate the conv across
    taps (start at tap 0, stop at tap 11); relus alternate Scalar/DVE.
  - fp8 (e4m3) DoubleRow matmuls (~1.44x at N=512) for a planned subset
    of conv (tap, chunk-pair) and proj (span, chunk-pair) items, chosen
    offline with an exact numerics model (errmodel/search2.py) against
    the 2e-2 max-rel-err gate. Late conv taps are cheap (error touches
    few spans); early proj spans are cheap (|h| grows ~sqrt(k)).
  - Scale-split quantization: conv fp8 operands are x*2^-3 and cw*2^3
    (product unscaled -> PSUM-compatible with fp16 taps; lifts the
    mostly-subnormal cw values into e4m3's normal range). proj weights
    are pw*2^4 in BOTH fp16 and fp8 copies; every span's bias-add is a
    DVE tensor_scalar (psum * 2^-4 + pb), exact for fp16 and fixing
    e4m3 subnormal waste for fp8.
  - h8 casts: relus always write fp16 h tiles (fast path); the fp8
    copies for fp8-proj spans are produced by GpSimd (otherwise idle)
    as tensor_scalar_sub(h - mu) into interleaved [P, 2, L] pair tiles.
    mu = ALPHA * sigma_span (exact per-d span sigma from conv_w) centers
    relu's one-sided distribution to cut e4m3 cast noise; the exact
    correction mu @ pw8 is folded into the per-span bias host-side.
  - Output is written feature-major ([W, D, L] in DRAM, host transposes
    to [L, W, D] - free for HW time), keeping every matmul at N=512.
"""

import os
import sys

import numpy as np

if True:  # make concourse importable regardless of harness cwd
    for _p in ("/opt/trn_rl_repo", "/opt/pypackages"):
        if _p not in sys.path and os.path.isdir(_p):
            sys.path.append(_p)

from contextlib import ExitStack  # noqa: E402

import ml_dtypes  # noqa: E402

import concourse.bacc as bacc  # noqa: E402
import concourse.bass as bass  # noqa: E402
import concourse.mybir as mybir  # noqa: E402
import concourse.tile as tile  # noqa: E402
from concourse import bass_utils  # noqa: E402

B, L, D, W = 8, 512, 768, 12
P = 128          # SBUF partitions
C = D // P       # 6 contraction chunks of 128
NP = C // 2      # 3 canonical chunk-pairs
LP = L + W - 1   # 523: right-padded sequence length

F32 = mybir.dt.float32
F16 = mybir.dt.float16
F8 = mybir.dt.float8e4
DR = mybir.MatmulPerfMode.DoubleRow
RELU = mybir.ActivationFunctionType.Relu

# ---- fp8 plan (from search2.py exact-model search) -------------------------
# tap/span -> tuple of canonical pair indices (pair g = chunks 2g, 2g+1)
CONV_FP8 = {9: (0, 1, 2), 10: (0, 1), 11: (0, 1, 2)}
PROJ_FP8 = {0: (0, 1, 2), 1: (0, 1, 2), 2: (0, 1, 2), 3: (0, 1), 4: (1,),
            5: (2,), 6: (1,), 7: (0,), 8: (1,)}
CONV_SCALE = 3     # xT8 = f8(x * 2^-3), cw8 = f8(cw * 2^3)
PROJ_SCALE = 4     # pw16/pw8 = pw * 2^4; bias op descales psum by 2^-4
ALPHA = 0.35       # h-centering: h8 = f8(h - ALPHA * sigma_span)
WARMUP = 42
SPLIT_FIRST = True
FP8_LOAD_AT = 5    # iteration that issues the xT8/cw8 bulk loads

# Knobs the test harness may flip before calling kernel():
TRACE = False
LAST_RESULTS = None


def _plan_key():
    return (tuple(sorted((k, tuple(v)) for k, v in CONV_FP8.items())),
            tuple(sorted((k, tuple(v)) for k, v in PROJ_FP8.items())),
            CONV_SCALE, PROJ_SCALE, ALPHA, WARMUP, SPLIT_FIRST, FP8_LOAD_AT)


def _conv8_items():
    return [(k, g) for k in sorted(CONV_FP8) for g in sorted(CONV_FP8[k])]


def _build_program() -> bass.Bass:
    conv8 = _conv8_items()
    cw8_idx = {kg: i for i, kg in enumerate(conv8)}
    any_proj8 = bool(PROJ_FP8)

    nc = bacc.Bacc(
        "TRN2",
        target_bir_lowering=False,
        debug=False,
        num_devices=B,
    )

    # DRAM I/O. Matmul inputs are pre-chunked host-side to [C, P, n] so each
    # chunk DMA is a clean 2D copy and compute can start on chunk 0 early.
    # All bulk operands are partition-major in DRAM so every load is one
    # long contiguous descriptor per partition (~9KB) - small gather
    # descriptors halve effective DMA bandwidth in the startup crunch.
    xT = nc.dram_tensor("xT", [P, C, LP], F16, kind="ExternalInput").ap()
    cw = nc.dram_tensor("cw", [W, P, C, D], F16, kind="ExternalInput").ap()
    pw = nc.dram_tensor("pw", [P, C, D], F16, kind="ExternalInput").ap()
    pb = nc.dram_tensor("pb", [P, W * C], F32, kind="ExternalInput").ap()
    if conv8:
        xT8 = nc.dram_tensor("xT8", [P, NP, 2, LP], F8, kind="ExternalInput").ap()
        cw8 = nc.dram_tensor("cw8", [P, len(conv8), 2, D], F8, kind="ExternalInput").ap()
    if any_proj8:
        pw8 = nc.dram_tensor("pw8", [P, NP, 2, D], F8, kind="ExternalInput").ap()
        mu = nc.dram_tensor("mu", [P, W * C], F32, kind="ExternalInput").ap()
    out = nc.dram_tensor("out", [W, D, L], F32, kind="ExternalOutput").ap()

    with tile.TileContext(nc) as tc, ExitStack() as ctx:
        const_pool = ctx.enter_context(tc.tile_pool(name="const", bufs=1))
        cw_pool = ctx.enter_context(tc.tile_pool(name="cw", bufs=2))
        h_pool = ctx.enter_context(tc.tile_pool(name="h", bufs=2))
        h8_pool = ctx.enter_context(tc.tile_pool(name="h8", bufs=2))
        out_pool = ctx.enter_context(tc.tile_pool(name="out", bufs=4))
        psc_pool = ctx.enter_context(tc.tile_pool(name="psc", bufs=1, space="PSUM"))
        psp_pool = ctx.enter_context(tc.tile_pool(name="psp", bufs=2, space="PSUM"))

        def dma_in(dst_ap, src_ap):
            nc.sync.dma_start(dst_ap, src_ap)

        if WARMUP:
            # Dependency-free matmuls keep the PE continuously busy from the
            # earliest possible moment so the HAM clock gate opens before the
            # real stream takes over.
            wa = const_pool.tile([P, P], F16, name="warm_a")
            nc.vector.memset(wa[:], 0.0)
            wp = psp_pool.tile([P, 512], F32, tag="psp", name="warm_ps")
            for wi in range(WARMUP):
                nc.tensor.matmul(
                    wp[:, 0:P], lhsT=wa[:], rhs=wa[:], start=True, stop=True
                )

        # --- startup loads, critical-path first -------------------------
        # The three tiny loads the first conv matmuls need go on the (empty)
        # Scalar queue; remaining startup loads are per-chunk DMAs on Sync in
        # compute-deadline order. pw/pb/pw8/mu ride the otherwise-idle Scalar
        # queue (first deadline ~21us: the span-0 proj).
        if SPLIT_FIRST:
            cw00a = cw_pool.tile([P, P], F16, tag="cw00a", name="cw00a")
            nc.scalar.dma_start(cw00a[:], cw[0, :, 0, 0:P])
            xt0 = const_pool.tile([P, LP], F16, tag="xt0", name="xt0")
            nc.scalar.dma_start(xt0[:], xT[:, 0, :])
            cw00b = cw_pool.tile([P, D - P], F16, tag="cw00b", name="cw00b")
            nc.scalar.dma_start(cw00b[:], cw[0, :, 0, P:D])
            first = 1
        else:
            first = 0
        cw0_c = [None] * C
        xt_c = [None] * C
        for c in range(first, C):
            t = cw_pool.tile([P, D], F16, tag=f"cw0_{c}", name=f"cw0_{c}")
            dma_in(t[:], cw[0, :, c, :])
            cw0_c[c] = t
            xt = const_pool.tile([P, LP], F16, tag=f"xt{c}", name=f"xt{c}")
            dma_in(xt[:], xT[:, c, :])
            xt_c[c] = xt

        def xt_ap(c):
            if SPLIT_FIRST and c == 0:
                return xt0[:]
            return xt_c[c][:]

        def tap_f16_chunks(k):
            cov = {c for g in CONV_FP8.get(k, ()) for c in (2 * g, 2 * g + 1)}
            return [c for c in range(C) if c not in cov]

        def load_cw(k):
            # Per-chunk DMAs into separate tiles: each rides its own DMA
            # channel (a single consolidated load is one channel and takes
            # ~8us - it stalled conv tap 1 by ~7us). Only the chunks the
            # fp16 matmuls actually need are loaded.
            tiles = {}
            for c in tap_f16_chunks(k):
                t = cw_pool.tile([P, D], F16, tag=f"cwt{c}", name=f"cw_{k}_{c}")
                dma_in(t[:], cw[k, :, c, :])
                tiles[c] = t
            return tiles

        cw_tiles = {}

        # cw tap 1's per-chunk loads go on Sync right after the tap-0 chunk
        # loads: with chunk-outer conv emission, conv tap 1's phase c only
        # needs cw1 chunk c - deadlines stagger with DMA arrival.
        if tap_f16_chunks(1):
            cw_tiles[1] = load_cw(1)

        pw_all = const_pool.tile([P, C, D], F16, tag="pw", name="pw")
        nc.scalar.dma_start(pw_all[:], pw[:, :, :])
        pw_t = [pw_all[:, c, :] for c in range(C)]
        pb_t = const_pool.tile([P, W * C], F32, name="pb")
        nc.scalar.dma_start(pb_t[:], pb[:])
        if any_proj8:
            pw8_t = const_pool.tile([P, NP, 2, D], F8, name="pw8")
            nc.scalar.dma_start(pw8_t[:], pw8[:, :, :, :])
            mu_t = const_pool.tile([P, W * C], F32, name="mu")
            nc.scalar.dma_start(mu_t[:], mu[:])
        if conv8:
            xT8_t = const_pool.tile([P, NP, 2, LP], F8, name="xT8")
            cw8_t = const_pool.tile([P, len(conv8), 2, D], F8, name="cw8")

            def load_fp8():
                dma_in(xT8_t[:], xT8[:, :, :, :])
                dma_in(cw8_t[:], cw8[:, :, :, :])

        def cw_slice(k, c, ob):
            """lhsT [P, 128] for conv tap k, contraction chunk c, out block ob."""
            if k == 0:
                if SPLIT_FIRST and c == 0:
                    if ob == 0:
                        return cw00a[:]
                    return cw00b[:, (ob - 1) * P: ob * P]
                return cw0_c[c][:, ob * P: (ob + 1) * P]
            return cw_tiles[k][c][:, ob * P: (ob + 1) * P]

        # 6 persistent PSUM banks accumulate the conv cumsum across taps.
        sp_acc = [
            psc_pool.tile([P, L], F32, tag=f"sp{ob}", name=f"sp{ob}")
            for ob in range(C)
        ]

        def fp8_cov(k):
            return {c: g for g in PROJ_FP8.get(k, ()) for c in (2 * g, 2 * g + 1)}

        def emit_post_chunk(k, ob, h_cur, h8_cur):
            cov = fp8_cov(k)
            if ob in cov:
                # Fused relu + centering + fp8 cast, straight from PSUM, on
                # the DVE: h8 = relu(psum) - mu. The fp16 copy of this chunk
                # is unused by this span's proj, so no separate relu needed.
                g = cov[ob]
                nc.vector.tensor_scalar(
                    out=h8_cur[g][:, (ob - 2 * g), :], in0=sp_acc[ob][:],
                    scalar1=0.0,
                    scalar2=mu_t[:, k * C + ob: k * C + ob + 1],
                    op0=mybir.AluOpType.max, op1=mybir.AluOpType.subtract,
                )
            elif len(cov) >= 2 or ob % 2 == 0:
                nc.scalar.activation(h_cur[ob][:], sp_acc[ob][:], RELU)
            else:
                nc.vector.tensor_scalar_max(h_cur[ob][:], sp_acc[ob][:], 0.0)

        def new_h_tiles(k):
            cov = fp8_cov(k)
            h_cur = [None if c in cov else
                     h_pool.tile([P, L], F16, tag=f"h{c}", name=f"h{c}_{k}")
                     for c in range(C)]
            h8_cur = {g: h8_pool.tile([P, 2, L], F8, tag=f"h8_{g}", name=f"h8_{g}_{k}")
                      for g in PROJ_FP8.get(k, ())}
            return h_cur, h8_cur

        def emit_conv_tap(k, h_cur, h8_cur):
            # Chunk-outer: contraction phase p touches only one cw chunk (or
            # fp8 pair), so tap k's matmuls start as soon as its FIRST chunk
            # lands and later chunks stream in behind the compute. Relus and
            # casts bunch at tap end - the lag-1 proj gives them a full conv
            # tap of slack.
            pairs8 = sorted(CONV_FP8.get(k, ()))
            f16c = tap_f16_chunks(k)
            phases = [("f", c) for c in f16c] + [("8", g) for g in pairs8]
            if k == 0:
                phases = sorted(phases, key=lambda t: t[1])  # chunk 0 first
            for idx, (t, v) in enumerate(phases):
                for ob in range(C):
                    first = (k == 0) and idx == 0
                    last = (k == W - 1) and idx == len(phases) - 1 and ob == C - 1
                    if t == "8":
                        nc.tensor.matmul(
                            sp_acc[ob][:],
                            lhsT=cw8_t[:, cw8_idx[(k, v)], :, ob * P:(ob + 1) * P],
                            rhs=xT8_t[:, v, :, k: k + L],
                            start=False, stop=last, perf_mode=DR,
                            skip_group_check=True,
                        )
                    else:
                        nc.tensor.matmul(
                            sp_acc[ob][:],
                            lhsT=cw_slice(k, v, ob),
                            rhs=xt_ap(v)[:, k: k + L],
                            start=first, stop=last,
                            skip_group_check=True,
                        )
            for ob in range(C):
                emit_post_chunk(k, ob, h_cur, h8_cur)

        def emit_proj(s, h_prev, h8_prev):
            pairs8 = sorted(PROJ_FP8.get(s, ()))
            cov = {c for g in pairs8 for c in (2 * g, 2 * g + 1)}
            f16c = [c for c in range(C) if c not in cov]
            for o2b in range(C):
                pp = psp_pool.tile([P, 512], F32, tag="psp", name=f"pp_{s}_{o2b}")
                ops = [("8", g) for g in pairs8] + [("f", c) for c in f16c]
                for idx, (t, v) in enumerate(ops):
                    if t == "8":
                        nc.tensor.matmul(
                            pp[:],
                            lhsT=pw8_t[:, v, :, o2b * P:(o2b + 1) * P],
                            rhs=h8_prev[v][:],
                            start=(idx == 0), stop=(idx == len(ops) - 1),
                            perf_mode=DR,
                        )
                    else:
                        nc.tensor.matmul(
                            pp[:],
                            lhsT=pw_t[v][:, o2b * P:(o2b + 1) * P],
                            rhs=h_prev[v][:],
                            start=(idx == 0), stop=(idx == len(ops) - 1),
                        )
                o_t = out_pool.tile([P, L], F32, tag="out", name=f"o_{s}_{o2b}")
                # Bias + 2^-PROJ_SCALE descale; split DVE/ACT to balance load
                # (Identity is in every ACT table set - no table reload).
                if o2b % 2 == 0:
                    nc.vector.tensor_scalar(
                        out=o_t[:], in0=pp[:],
                        scalar1=2.0 ** -PROJ_SCALE,
                        scalar2=pb_t[:, s * C + o2b: s * C + o2b + 1],
                        op0=mybir.AluOpType.mult, op1=mybir.AluOpType.add,
                    )
                else:
                    nc.scalar.activation(
                        o_t[:], pp[:], mybir.ActivationFunctionType.Identity,
                        bias=pb_t[:, s * C + o2b: s * C + o2b + 1],
                        scale=2.0 ** -PROJ_SCALE,
                    )
                nc.sync.dma_start(out[s, o2b * P:(o2b + 1) * P, :], o_t[:])

        # ---- tap 0 (chunk-outer like every tap) -------------------------
        h_cur, h8_cur = new_h_tiles(0)
        emit_conv_tap(0, h_cur, h8_cur)

        # ---- main lagged loop ------------------------------------------
        h_prev, h8_prev = h_cur, h8_cur
        for k in range(1, W):
            if k + 1 < W and tap_f16_chunks(k + 1):
                cw_tiles[k + 1] = load_cw(k + 1)
            if k == FP8_LOAD_AT and conv8:
                load_fp8()
            h_cur, h8_cur = new_h_tiles(k)
            emit_conv_tap(k, h_cur, h8_cur)
            emit_proj(k - 1, h_prev, h8_prev)
            h_prev, h8_prev = h_cur, h8_cur
        emit_proj(W - 1, h_prev, h8_prev)

    nc.compile()
    return nc


_program_cache: dict = {}


def _get_program() -> bass.Bass:
    key = _plan_key()
    if key not in _program_cache:
        _program_cache[key] = _build_program()
    return _program_cache[key]


def _prep_inputs(x, conv_w, proj_w, proj_b):
    x = np.asarray(x, dtype=np.float32)
    conv_w = np.asarray(conv_w, dtype=np.float32)
    proj_w = np.asarray(proj_w, dtype=np.float32)
    proj_b = np.asarray(proj_b, dtype=np.float32)
    f8 = ml_dtypes.float8_e4m3

    xT_f32 = np.zeros((B, D, LP), dtype=np.float32)              # [B, D, L+W-1]
    xT_f32[:, :, :L] = x.transpose(0, 2, 1)
    xT_f32 = xT_f32.reshape(B, C, P, LP)
    xT16 = np.ascontiguousarray(xT_f32.transpose(0, 2, 1, 3).astype(np.float16))
    cwT_f32 = conv_w.transpose(2, 1, 0).reshape(W, C, P, D)      # [W, C, P, o]
    cw16 = np.ascontiguousarray(cwT_f32.transpose(0, 2, 1, 3).astype(np.float16))
    pw_f32 = proj_w.T.reshape(C, P, D) * 2.0 ** PROJ_SCALE
    pw16 = np.ascontiguousarray(pw_f32.transpose(1, 0, 2).astype(np.float16))

    maps = {"xT": xT16, "cw": cw16, "pw": pw16}
    per_b = {"xT"}

    conv8 = _conv8_items()
    if conv8:
        # Pair layout [g, P, 2, n] feeds DoubleRow matmuls (contract 2
        # k-chunks per instruction). Scale-split: product is unscaled.
        maps["xT8"] = np.ascontiguousarray(
            (xT_f32 * 2.0 ** -CONV_SCALE)
            .reshape(B, NP, 2, P, LP).transpose(0, 3, 1, 2, 4).astype(f8))
        per_b.add("xT8")
        cw8 = np.stack([
            (cwT_f32[k, 2 * g: 2 * g + 2] * 2.0 ** CONV_SCALE).transpose(1, 0, 2)
            for (k, g) in conv8
        ])  # [n_items, P, 2, D]
        maps["cw8"] = np.ascontiguousarray(cw8.transpose(1, 0, 2, 3).astype(f8))

    # per-span bias, with the h-centering correction folded in
    pb_full = np.broadcast_to(proj_b, (W, D)).copy()             # [W, D(o)]
    if PROJ_FP8:
        pw8_pairs = pw_f32.reshape(NP, 2, P, D).transpose(0, 2, 1, 3)
        pw8_q = pw8_pairs.astype(f8)
        maps["pw8"] = np.ascontiguousarray(pw8_q.transpose(1, 0, 2, 3))
        # sigma of span s per input-dim d (x ~ iid N(0,1) exactly known)
        sig = np.sqrt(np.cumsum((conv_w ** 2).sum(axis=1), axis=-1)).T  # [W, D]
        mu_f = ALPHA * sig                                        # [W, D_in]
        mu_t = np.zeros((P, W * C), np.float32)
        pw8_f32 = (pw8_q.astype(np.float32)
                   .transpose(0, 2, 1, 3).reshape(D, D)) * 2.0 ** -PROJ_SCALE
        for s, pairs in PROJ_FP8.items():
            chunks = [c for g in pairs for c in (2 * g, 2 * g + 1)]
            for c in chunks:
                mu_t[:, s * C + c] = mu_f[s, c * P:(c + 1) * P]
            dmask = np.zeros(D, np.float32)
            for c in chunks:
                dmask[c * P:(c + 1) * P] = 1.0
            pb_full[s] += (mu_f[s] * dmask) @ pw8_f32
        maps["mu"] = np.ascontiguousarray(mu_t)
    pb_t = np.zeros((P, W * C), np.float32)
    for s in range(W):
        pb_t[:, s * C:(s + 1) * C] = pb_full[s].reshape(C, P).T
    maps["pb"] = np.ascontiguousarray(pb_t)
    return maps, per_b


def kernel(x, conv_w, proj_w, proj_b):
    global LAST_RESULTS
    nc = _get_program()
    maps, per_b = _prep_inputs(x, conv_w, proj_w, proj_b)
    in_maps = []
    for b in range(B):
        in_maps.append({k: (v[b] if k in per_b else v) for k, v in maps.items()})
    res = bass_utils.run_bass_kernel_spmd(
        nc, in_maps, core_ids=list(range(B)), trace=TRACE
    )
    LAST_RESULTS = res
    # per-core out is [W, D, L]; final layout is [L, W, D]
    return np.stack(
        [np.ascontiguousarray(r["out"].transpose(2, 0, 1)) for r in res.results],
        axis=0,
    )


# revision 12
# speedup vs baseline: 1.2174x; 1.0243x over previous
"""Trainium2 Bass kernel for nn_ConvShare: multi-width causal conv + shared projection.

Reference computation (per batch element b):
    xpad = pad(x[b], L -> L+W-1)                       # [L+11, D]
    taps[k]  = xpad[k:k+L, :] @ conv_w[:, :, k].T      # [L, D], k = 0..W-1
    spans[k] = cumsum_k taps                           # [L, D]
    h[k]     = relu(spans[k])
    out[:, k, :] = h[k] @ proj_w.T + proj_b            # [L, W, D]

Sharding: data-parallel over batch B=8 across the 8 NeuronCores (no
communication; conv_w/proj_w replicated per core).

The kernel is PE-bound: 2 * W * L * D * D = 7.25 GMAC/core = 864 N=512
fp16 matmuls ~= 186.5 us at the warm 2.4 GHz rate. Design:

  - WARMUP dependency-free N=128 matmuls open the HAM clock gate
    (1.2 -> 2.4 GHz after ~3.4us of continuous PE busy) while startup
    DMAs land; startup load choreography unchanged from the tuned
    baseline (split first cw chunk + xt0 on the Scalar queue, per-chunk
    Sync loads in deadline order, pw/pb on Scalar).
  - LAGGED PROJ pipeline: proj of span k-1 is emitted between conv tap
    k and conv tap k+1, so every proj's relu (and fp8 cast) inputs are
    produced a full conv-tap (~6us) earlier - no relu/cast latency can
    ever bubble the PE stream.
  - PSUM cumsum: 6 persistent PSUM banks accumulate the conv across
    taps (start at tap 0, stop at tap 11); relus alternate Scalar/DVE.
  - fp8 (e4m3) DoubleRow matmuls (~1.44x at N=512) for a planned subset
    of conv (tap, chunk-pair) and proj (span, chunk-pair) items, chosen
    offline with an exact numerics model (errmodel/search2.py) against
    the 2e-2 max-rel-err gate. Late conv taps are cheap (error touches
    few spans); early proj spans are cheap (|h| grows ~sqrt(k)).
  - Scale-split quantization: conv fp8 operands are x*2^-3 and cw*2^3
    (product unscaled -> PSUM-compatible with fp16 taps; lifts the
    mostly-subnormal cw values into e4m3's normal range). proj weights
    are pw*2^4 in BOTH fp16 and fp8 copies; every span's bias-add is a
    DVE tensor_scalar (psum * 2^-4 + pb), exact for fp16 and fixing
    e4m3 subnormal waste for fp8.
  - h8 casts: relus always write fp16 h tiles (fast path); the fp8
    copies for fp8-proj spans are produced by GpSimd (otherwise idle)
    as tensor_scalar_sub(h - mu) into interleaved [P, 2, L] pair tiles.
    mu = ALPHA * sigma_span (exact per-d span sigma from conv_w) centers
    relu's one-sided distribution to cut e4m3 cast noise; the exact
    correction mu @ pw8 is folded into the per-span bias host-side.
  - Output is written feature-major ([W, D, L] in DRAM, host transposes
    to [L, W, D] - free for HW time), keeping every matmul at N=512.
"""

import os
import sys

import numpy as np

if True:  # make concourse importable regardless of harness cwd
    for _p in ("/opt/trn_rl_repo", "/opt/pypackages"):
        if _p not in sys.path and os.path.isdir(_p):
            sys.path.append(_p)

from contextlib import ExitStack  # noqa: E402

import ml_dtypes  # noqa: E402

import concourse.bacc as bacc  # noqa: E402
import concourse.bass as bass  # noqa: E402
import concourse.mybir as mybir  # noqa: E402
import concourse.tile as tile  # noqa: E402
from concourse import bass_utils  # noqa: E402

B, L, D, W = 8, 512, 768, 12
P = 128          # SBUF partitions
C = D // P       # 6 contraction chunks of 128
NP = C // 2      # 3 canonical chunk-pairs
LP = L + W - 1   # 523: right-padded sequence length

F32 = mybir.dt.float32
F16 = mybir.dt.float16
F8 = mybir.dt.float8e4
DR = mybir.MatmulPerfMode.DoubleRow
RELU = mybir.ActivationFunctionType.Relu

# ---- fp8 plan (from search2.py exact-model search) -------------------------
# tap/span -> tuple of canonical pair indices (pair g = chunks 2g, 2g+1)
CONV_FP8 = {9: (0, 1, 2), 10: (0, 1), 11: (0, 1, 2)}
PROJ_FP8 = {0: (0, 1, 2), 1: (0, 1, 2), 2: (0, 1, 2), 3: (0, 1, 2), 4: (0, 2),
            5: (2,), 6: (1,), 7: (0,), 8: (1,)}
CONV_SCALE = 3     # xT8 = f8(x * 2^-3), cw8 = f8(cw * 2^3)
PROJ_SCALE = 4     # pw16/pw8 = pw * 2^4; bias op descales psum by 2^-4
# h-centering coefficient per span: h8 = f8(h - alpha_s * sigma_span)
ALPHA = {0: 0.35, 1: 0.35, 2: 0.35, 3: 0.5, 4: 0.4, 5: 0.35, 6: 0.35,
         7: 0.35, 8: 0.35}
WARMUP = 34
SPLIT_FIRST = True
FP8_LOAD_AT = 5    # iteration that issues the xT8/cw8 bulk loads

# Knobs the test harness may flip before calling kernel():
TRACE = False
LAST_RESULTS = None


def _plan_key():
    return (tuple(sorted((k, tuple(v)) for k, v in CONV_FP8.items())),
            tuple(sorted((k, tuple(v)) for k, v in PROJ_FP8.items())),
            CONV_SCALE, PROJ_SCALE, tuple(sorted(ALPHA.items())),
            WARMUP, SPLIT_FIRST, FP8_LOAD_AT)


def _conv8_items():
    return [(k, g) for k in sorted(CONV_FP8) for g in sorted(CONV_FP8[k])]


def _build_program() -> bass.Bass:
    conv8 = _conv8_items()
    cw8_idx = {kg: i for i, kg in enumerate(conv8)}
    any_proj8 = bool(PROJ_FP8)

    nc = bacc.Bacc(
        "TRN2",
        target_bir_lowering=False,
        debug=False,
        num_devices=B,
    )

    # DRAM I/O. Matmul inputs are pre-chunked host-side to [C, P, n] so each
    # chunk DMA is a clean 2D copy and compute can start on chunk 0 early.
    # All bulk operands are partition-major in DRAM so every load is one
    # long contiguous descriptor per partition (~9KB) - small gather
    # descriptors halve effective DMA bandwidth in the startup crunch.
    xT = nc.dram_tensor("xT", [P, C, LP], F16, kind="ExternalInput").ap()
    cw = nc.dram_tensor("cw", [W, P, C, D], F16, kind="ExternalInput").ap()
    pw = nc.dram_tensor("pw", [P, C, D], F16, kind="ExternalInput").ap()
    pb = nc.dram_tensor("pb", [P, W * C], F32, kind="ExternalInput").ap()
    if conv8:
        xT8 = nc.dram_tensor("xT8", [P, NP, 2, LP], F8, kind="ExternalInput").ap()
        cw8 = nc.dram_tensor("cw8", [P, len(conv8), 2, D], F8, kind="ExternalInput").ap()
    if any_proj8:
        pw8 = nc.dram_tensor("pw8", [P, NP, 2, D], F8, kind="ExternalInput").ap()
        mu = nc.dram_tensor("mu", [P, W * C], F32, kind="ExternalInput").ap()
    out = nc.dram_tensor("out", [W, D, L], F32, kind="ExternalOutput").ap()

    with tile.TileContext(nc) as tc, ExitStack() as ctx:
        const_pool = ctx.enter_context(tc.tile_pool(name="const", bufs=1))
        cw_pool = ctx.enter_context(tc.tile_pool(name="cw", bufs=2))
        h_pool = ctx.enter_context(tc.tile_pool(name="h", bufs=2))
        h8_pool = ctx.enter_context(tc.tile_pool(name="h8", bufs=2))
        out_pool = ctx.enter_context(tc.tile_pool(name="out", bufs=4))
        psc_pool = ctx.enter_context(tc.tile_pool(name="psc", bufs=1, space="PSUM"))
        psp_pool = ctx.enter_context(tc.tile_pool(name="psp", bufs=2, space="PSUM"))

        def dma_in(dst_ap, src_ap):
            nc.sync.dma_start(dst_ap, src_ap)

        if WARMUP:
            # Dependency-free matmuls keep the PE continuously busy from the
            # earliest possible moment so the HAM clock gate opens before the
            # real stream takes over.
            wa = const_pool.tile([P, P], F16, name="warm_a")
            nc.vector.memset(wa[:], 0.0)
            wp = psp_pool.tile([P, 512], F32, tag="psp", name="warm_ps")
            for wi in range(WARMUP):
                nc.tensor.matmul(
                    wp[:, 0:P], lhsT=wa[:], rhs=wa[:], start=True, stop=True
                )

        # --- startup loads, critical-path first -------------------------
        # The three tiny loads the first conv matmuls need go on the (empty)
        # Scalar queue; remaining startup loads are per-chunk DMAs on Sync in
        # compute-deadline order. pw/pb/pw8/mu ride the otherwise-idle Scalar
        # queue (first deadline ~21us: the span-0 proj).
        if SPLIT_FIRST:
            cw00a = cw_pool.tile([P, P], F16, tag="cw00a", name="cw00a")
            nc.scalar.dma_start(cw00a[:], cw[0, :, 0, 0:P])
            xt0 = const_pool.tile([P, LP], F16, tag="xt0", name="xt0")
            nc.scalar.dma_start(xt0[:], xT[:, 0, :])
            cw00b = cw_pool.tile([P, D - P], F16, tag="cw00b", name="cw00b")
            nc.scalar.dma_start(cw00b[:], cw[0, :, 0, P:D])
            first = 1
        else:
            first = 0
        cw0_c = [None] * C
        xt_c = [None] * C
        for c in range(first, C):
            t = cw_pool.tile([P, D], F16, tag=f"cw0_{c}", name=f"cw0_{c}")
            dma_in(t[:], cw[0, :, c, :])
            cw0_c[c] = t
            xt = const_pool.tile([P, LP], F16, tag=f"xt{c}", name=f"xt{c}")
            dma_in(xt[:], xT[:, c, :])
            xt_c[c] = xt

        def xt_ap(c):
            if SPLIT_FIRST and c == 0:
                return xt0[:]
            return xt_c[c][:]

        def tap_f16_chunks(k):
            cov = {c for g in CONV_FP8.get(k, ()) for c in (2 * g, 2 * g + 1)}
            return [c for c in range(C) if c not in cov]

        def load_cw(k):
            # Per-chunk DMAs into separate tiles: each rides its own DMA
            # channel (a single consolidated load is one channel and takes
            # ~8us - it stalled conv tap 1 by ~7us). Only the chunks the
            # fp16 matmuls actually need are loaded.
            tiles = {}
            for c in tap_f16_chunks(k):
                t = cw_pool.tile([P, D], F16, tag=f"cwt{c}", name=f"cw_{k}_{c}")
                dma_in(t[:], cw[k, :, c, :])
                tiles[c] = t
            return tiles

        cw_tiles = {}

        # cw tap 1's per-chunk loads go on Sync right after the tap-0 chunk
        # loads: with chunk-outer conv emission, conv tap 1's phase c only
        # needs cw1 chunk c - deadlines stagger with DMA arrival.
        if tap_f16_chunks(1):
            cw_tiles[1] = load_cw(1)

        pw_all = const_pool.tile([P, C, D], F16, tag="pw", name="pw")
        nc.scalar.dma_start(pw_all[:], pw[:, :, :])
        pw_t = [pw_all[:, c, :] for c in range(C)]
        pb_t = const_pool.tile([P, W * C], F32, name="pb")
        nc.scalar.dma_start(pb_t[:], pb[:])
        if any_proj8:
            pw8_t = const_pool.tile([P, NP, 2, D], F8, name="pw8")
            nc.scalar.dma_start(pw8_t[:], pw8[:, :, :, :])
            mu_t = const_pool.tile([P, W * C], F32, name="mu")
            nc.scalar.dma_start(mu_t[:], mu[:])
        if conv8:
            xT8_t = const_pool.tile([P, NP, 2, LP], F8, name="xT8")
            cw8_t = const_pool.tile([P, len(conv8), 2, D], F8, name="cw8")

            def load_fp8():
                dma_in(xT8_t[:], xT8[:, :, :, :])
                dma_in(cw8_t[:], cw8[:, :, :, :])

        def cw_slice(k, c, ob):
            """lhsT [P, 128] for conv tap k, contraction chunk c, out block ob."""
            if k == 0:
                if SPLIT_FIRST and c == 0:
                    if ob == 0:
                        return cw00a[:]
                    return cw00b[:, (ob - 1) * P: ob * P]
                return cw0_c[c][:, ob * P: (ob + 1) * P]
            return cw_tiles[k][c][:, ob * P: (ob + 1) * P]

        # 6 persistent PSUM banks accumulate the conv cumsum across taps.
        sp_acc = [
            psc_pool.tile([P, L], F32, tag=f"sp{ob}", name=f"sp{ob}")
            for ob in range(C)
        ]

        def fp8_cov(k):
            return {c: g for g in PROJ_FP8.get(k, ()) for c in (2 * g, 2 * g + 1)}

        def emit_post_chunk(k, ob, h_cur, h8_cur):
            cov = fp8_cov(k)
            if ob in cov:
                # Fused relu + centering + fp8 cast, straight from PSUM, on
                # the DVE: h8 = relu(psum) - mu. The fp16 copy of this chunk
                # is unused by this span's proj, so no separate relu needed.
                g = cov[ob]
                nc.vector.tensor_scalar(
                    out=h8_cur[g][:, (ob - 2 * g), :], in0=sp_acc[ob][:],
                    scalar1=0.0,
                    scalar2=mu_t[:, k * C + ob: k * C + ob + 1],
                    op0=mybir.AluOpType.max, op1=mybir.AluOpType.subtract,
                )
            elif len(cov) >= 2 or ob % 2 == 0:
                nc.scalar.activation(h_cur[ob][:], sp_acc[ob][:], RELU)
            else:
                nc.vector.tensor_scalar_max(h_cur[ob][:], sp_acc[ob][:], 0.0)

        def new_h_tiles(k):
            cov = fp8_cov(k)
            h_cur = [None if c in cov else
                     h_pool.tile([P, L], F16, tag=f"h{c}", name=f"h{c}_{k}")
                     for c in range(C)]
            h8_cur = {g: h8_pool.tile([P, 2, L], F8, tag=f"h8_{g}", name=f"h8_{g}_{k}")
                      for g in PROJ_FP8.get(k, ())}
            return h_cur, h8_cur

        def emit_conv_tap(k, h_cur, h8_cur):
            # Chunk-outer: contraction phase p touches only one cw chunk (or
            # fp8 pair), so tap k's matmuls start as soon as its FIRST chunk
            # lands and later chunks stream in behind the compute. Relus and
            # casts bunch at tap end - the lag-1 proj gives them a full conv
            # tap of slack.
            pairs8 = sorted(CONV_FP8.get(k, ()))
            f16c = tap_f16_chunks(k)
            phases = [("f", c) for c in f16c] + [("8", g) for g in pairs8]
            if k == 0:
                phases = sorted(phases, key=lambda t: t[1])  # chunk 0 first
            for idx, (t, v) in enumerate(phases):
                for ob in range(C):
                    first = (k == 0) and idx == 0
                    last = (k == W - 1) and idx == len(phases) - 1 and ob == C - 1
                    if t == "8":
                        nc.tensor.matmul(
                            sp_acc[ob][:],
                            lhsT=cw8_t[:, cw8_idx[(k, v)], :, ob * P:(ob + 1) * P],
                            rhs=xT8_t[:, v, :, k: k + L],
                            start=False, stop=last, perf_mode=DR,
                            skip_group_check=True,
                        )
                    else:
                        nc.tensor.matmul(
                            sp_acc[ob][:],
                            lhsT=cw_slice(k, v, ob),
                            rhs=xt_ap(v)[:, k: k + L],
                            start=first, stop=last,
                            skip_group_check=True,
                        )
            for ob in range(C):
                emit_post_chunk(k, ob, h_cur, h8_cur)

        def emit_proj(s, h_prev, h8_prev):
            pairs8 = sorted(PROJ_FP8.get(s, ()))
            cov = {c for g in pairs8 for c in (2 * g, 2 * g + 1)}
            f16c = [c for c in range(C) if c not in cov]
            for o2b in range(C):
                pp = psp_pool.tile([P, 512], F32, tag="psp", name=f"pp_{s}_{o2b}")
                ops = [("8", g) for g in pairs8] + [("f", c) for c in f16c]
                for idx, (t, v) in enumerate(ops):
                    if t == "8":
                        nc.tensor.matmul(
                            pp[:],
                            lhsT=pw8_t[:, v, :, o2b * P:(o2b + 1) * P],
                            rhs=h8_prev[v][:],
                            start=(idx == 0), stop=(idx == len(ops) - 1),
                            perf_mode=DR,
                        )
                    else:
                        nc.tensor.matmul(
                            pp[:],
                            lhsT=pw_t[v][:, o2b * P:(o2b + 1) * P],
                            rhs=h_prev[v][:],
                            start=(idx == 0), stop=(idx == len(ops) - 1),
                        )
                o_t = out_pool.tile([P, L], F32, tag="out", name=f"o_{s}_{o2b}")
                # Bias + 2^-PROJ_SCALE descale; split DVE/ACT to balance load
                # (Identity is in every ACT table set - no table reload).
                if o2b % 2 == 0:
                    nc.vector.tensor_scalar(
                        out=o_t[:], in0=pp[:],
                        scalar1=2.0 ** -PROJ_SCALE,
                        scalar2=pb_t[:, s * C + o2b: s * C + o2b + 1],
                        op0=mybir.AluOpType.mult, op1=mybir.AluOpType.add,
                    )
                else:
                    nc.scalar.activation(
                        o_t[:], pp[:], mybir.ActivationFunctionType.Identity,
                        bias=pb_t[:, s * C + o2b: s * C + o2b + 1],
                        scale=2.0 ** -PROJ_SCALE,
                    )
                nc.sync.dma_start(out[s, o2b * P:(o2b + 1) * P, :], o_t[:])

        # ---- tap 0 (chunk-outer like every tap) -------------------------
        h_cur, h8_cur = new_h_tiles(0)
        emit_conv_tap(0, h_cur, h8_cur)

        # ---- main lagged loop ------------------------------------------
        h_prev, h8_prev = h_cur, h8_cur
        for k in range(1, W):
            if k + 1 < W and tap_f16_chunks(k + 1):
                cw_tiles[k + 1] = load_cw(k + 1)
            if k == FP8_LOAD_AT and conv8:
                load_fp8()
            h_cur, h8_cur = new_h_tiles(k)
            emit_conv_tap(k, h_cur, h8_cur)
            emit_proj(k - 1, h_prev, h8_prev)
            h_prev, h8_prev = h_cur, h8_cur
        emit_proj(W - 1, h_prev, h8_prev)

    nc.compile()
    return nc


_program_cache: dict = {}


def _get_program() -> bass.Bass:
    key = _plan_key()
    if key not in _program_cache:
        _program_cache[key] = _build_program()
    return _program_cache[key]


def _prep_inputs(x, conv_w, proj_w, proj_b):
    x = np.asarray(x, dtype=np.float32)
    conv_w = np.asarray(conv_w, dtype=np.float32)
    proj_w = np.asarray(proj_w, dtype=np.float32)
    proj_b = np.asarray(proj_b, dtype=np.float32)
    f8 = ml_dtypes.float8_e4m3

    xT_f32 = np.zeros((B, D, LP), dtype=np.float32)              # [B, D, L+W-1]
    xT_f32[:, :, :L] = x.transpose(0, 2, 1)
    xT_f32 = xT_f32.reshape(B, C, P, LP)
    xT16 = np.ascontiguousarray(xT_f32.transpose(0, 2, 1, 3).astype(np.float16))
    cwT_f32 = conv_w.transpose(2, 1, 0).reshape(W, C, P, D)      # [W, C, P, o]
    cw16 = np.ascontiguousarray(cwT_f32.transpose(0, 2, 1, 3).astype(np.float16))
    pw_f32 = proj_w.T.reshape(C, P, D) * 2.0 ** PROJ_SCALE
    pw16 = np.ascontiguousarray(pw_f32.transpose(1, 0, 2).astype(np.float16))

    maps = {"xT": xT16, "cw": cw16, "pw": pw16}
    per_b = {"xT"}

    conv8 = _conv8_items()
    if conv8:
        # Pair layout [g, P, 2, n] feeds DoubleRow matmuls (contract 2
        # k-chunks per instruction). Scale-split: product is unscaled.
        maps["xT8"] = np.ascontiguousarray(
            (xT_f32 * 2.0 ** -CONV_SCALE)
            .reshape(B, NP, 2, P, LP).transpose(0, 3, 1, 2, 4).astype(f8))
        per_b.add("xT8")
        cw8 = np.stack([
            (cwT_f32[k, 2 * g: 2 * g + 2] * 2.0 ** CONV_SCALE).transpose(1, 0, 2)
            for (k, g) in conv8
        ])  # [n_items, P, 2, D]
        maps["cw8"] = np.ascontiguousarray(cw8.transpose(1, 0, 2, 3).astype(f8))

    # per-span bias, with the h-centering correction folded in
    pb_full = np.broadcast_to(proj_b, (W, D)).copy()             # [W, D(o)]
    if PROJ_FP8:
        pw8_pairs = pw_f32.reshape(NP, 2, P, D).transpose(0, 2, 1, 3)
        pw8_q = pw8_pairs.astype(f8)
        maps["pw8"] = np.ascontiguousarray(pw8_q.transpose(1, 0, 2, 3))
        # sigma of span s per input-dim d (x ~ iid N(0,1) exactly known)
        sig = np.sqrt(np.cumsum((conv_w ** 2).sum(axis=1), axis=-1)).T  # [W, D]
        alphas = np.array([ALPHA.get(s, 0.35) for s in range(W)], np.float32)
        mu_f = alphas[:, None] * sig                              # [W, D_in]
        mu_t = np.zeros((P, W * C), np.float32)
        pw8_f32 = (pw8_q.astype(np.float32)
                   .transpose(0, 2, 1, 3).reshape(D, D)) * 2.0 ** -PROJ_SCALE
        for s, pairs in PROJ_FP8.items():
            chunks = [c for g in pairs for c in (2 * g, 2 * g + 1)]
            for c in chunks:
                mu_t[:, s * C + c] = mu_f[s, c * P:(c + 1) * P]
            dmask = np.zeros(D, np.float32)
            for c in chunks:
                dmask[c * P:(c + 1) * P] = 1.0
            pb_full[s] += (mu_f[s] * dmask) @ pw8_f32
        maps["mu"] = np.ascontiguousarray(mu_t)
    pb_t = np.zeros((P, W * C), np.float32)
    for s in range(W):
        pb_t[:, s * C:(s + 1) * C] = pb_full[s].reshape(C, P).T
    maps["pb"] = np.ascontiguousarray(pb_t)
    return maps, per_b


def kernel(x, conv_w, proj_w, proj_b):
    global LAST_RESULTS
    nc = _get_program()
    maps, per_b = _prep_inputs(x, conv_w, proj_w, proj_b)
    in_maps = []
    for b in range(B):
        in_maps.append({k: (v[b] if k in per_b else v) for k, v in maps.items()})
    res = bass_utils.run_bass_kernel_spmd(
        nc, in_maps, core_ids=list(range(B)), trace=TRACE
    )
    LAST_RESULTS = res
    # per-core out is [W, D, L]; final layout is [L, W, D]
    return np.stack(
        [np.ascontiguousarray(r["out"].transpose(2, 0, 1)) for r in res.results],
        axis=0,
    )
